# revision 18
# baseline (speedup 1.0000x reference)
"""MultiHeadAttention Trainium2 kernel (8 NeuronCores).

Sharding: core c -> (batch b = c//2, head-group g = c%2) of the 12 heads.
Each core computes attention for its 6 heads of one batch element and a
partial projection; the host sums the two head-group partials per batch
element and adds proj bias.

Per-core dataflow (feat-major / transposed layouts to avoid transposes):
  x [S,768] --PE-transpose--> xT [768,S]
  qT/kT = (wqk^T @ xT) + bias     (float32r matmuls, TF32-class)
  v [S,384] seq-major (+ ones column for softmax denominators)
  scoresT[sk,sq] = kT_chunk^T-pair @ qT  (2 heads packed in PE row groups)
  pT = exp(scoresT/8)              (ScalarE, no max subtraction - bounded)
  avT[65,sq] = [v|1]^T @ pT        (row 64 = softmax denominator)
  attn_outT = avT[0:64] * (1/avT[64]) broadcast via K=1 PE matmul
  yT[768,S] = wp^T @ attn_outT     (partial projection, host sums pairs)
"""
import sys

sys.path.insert(0, "/opt/trn_rl_repo")

import numpy as np

import concourse.bass as bass
import concourse.mybir as mybir
import concourse.tile as tile
from concourse import bacc
from concourse.bass_utils import run_bass_kernel_spmd
from concourse.masks import make_identity

F32 = mybir.dt.float32
F32R = mybir.dt.float32r
EXP = mybir.ActivationFunctionType.Exp
ADD = mybir.AluOpType.add

HID = 768
D = 64  # head dim
LHEADS = 6  # heads per core
PAIRS = 3


def build_nc(S: int, taps: bool = False):
    nc = bacc.Bacc("TRN2", target_bir_lowering=False, debug=False)
    NSEQ = S // 128  # seq chunks of 128
    NBLK = S // 512  # seq blocks of 512
    XG = 4  # x DMA chunk groups
    NXG = NSEQ // XG

    x = nc.dram_tensor("x", [S, HID], F32, kind="ExternalInput")
    wqk = nc.dram_tensor("wqk", [HID, 768], F32, kind="ExternalInput")
    wv = nc.dram_tensor("wv", [HID, 384], F32, kind="ExternalInput")
    bqk = nc.dram_tensor("bqk", [768], F32, kind="ExternalInput")
    bv = nc.dram_tensor("bv", [384], F32, kind="ExternalInput")
    wp = nc.dram_tensor("wp", [384, HID], F32, kind="ExternalInput")
    yT = nc.dram_tensor("yT", [HID, S], F32, kind="ExternalOutput")
    if taps:
        t_xT = nc.dram_tensor("t_xT", [128, 6 * S], F32, kind="ExternalOutput")
        t_qT = nc.dram_tensor("t_qT", [128, S], F32, kind="ExternalOutput")
        t_kT = nc.dram_tensor("t_kT", [128, S], F32, kind="ExternalOutput")
        t_v = nc.dram_tensor("t_v", [128, S // 128 * 2 * 65], F32, kind="ExternalOutput")
        t_pt = nc.dram_tensor("t_pt", [128, 1024], F32, kind="ExternalOutput")
        t_av = nc.dram_tensor("t_av", [65, 512], F32, kind="ExternalOutput")
        t_ao = nc.dram_tensor("t_ao", [128, 3 * S], F32, kind="ExternalOutput")
        t_rc = nc.dram_tensor("t_rc", [1, 512], F32, kind="ExternalOutput")
        t_bc = nc.dram_tensor("t_bc", [D, 512], F32, kind="ExternalOutput")

    with tile.TileContext(nc) as tc:
        with (
            tc.tile_pool(name="const", bufs=1) as cp,
            tc.tile_pool(name="wts", bufs=1) as wpool,
            tc.tile_pool(name="ao", bufs=1) as aop,
            tc.tile_pool(name="ps", bufs=2, space="PSUM") as ps,
        ):
            ident = cp.tile([128, 128], F32, tag="ident")
            make_identity(nc, ident[:])
            ones_f = cp.tile([33, 128], F32, tag="onesf")
            nc.vector.memset(ones_f[:], 1.0)
            ones_r = cp.tile([33, 128], F32R, tag="ones")
            nc.vector.tensor_copy(ones_r[:], ones_f[:])
            bqk_sb = cp.tile([128, 6], F32, tag="bqk")
            nc.sync.dma_start(bqk_sb[:], bqk[:].rearrange("(c p) -> p c", p=128))
            bv_sb = cp.tile([1, 384], F32, tag="bvs")
            nc.sync.dma_start(bv_sb[:], bv[:].rearrange("(o f) -> o f", o=1))
            bv_r = cp.tile([1, 384], F32R, tag="bvr")
            nc.vector.tensor_copy(bv_r[:], bv_sb[:])

            aoT = aop.tile([128, PAIRS, S], F32R, tag="aoT")

            with tc.tile_pool(name="xT", bufs=1) as xtp:
                xT = xtp.tile([128, 6, S], F32R, tag="xT")

                # --- x DMA (group 0 first), weights, transposes, qkT(0), v interleaved ---
                with tc.tile_pool(name="wstage", bufs=1) as wst, \
                     tc.tile_pool(name="xin", bufs=1) as xin:
                    x_ap = x[:].rearrange("(n p) d -> p n d", p=128)
                    x_ts = []
                    for g in range(XG):
                        x_t = xin.tile([128, NXG, HID], F32, tag=f"x{g}", name=f"x_t{g}")
                        x_ts.append(x_t)
                    nc.sync.dma_start(x_ts[0][:], x_ap[:, 0:NXG, :])
                    wqk_f = wst.tile([128, 6, 768], F32, tag="wqkf")
                    wqk_ap = wqk[:].rearrange("(c p) f -> p c f", p=128)
                    for kc in range(6):
                        nc.sync.dma_start(
                            wqk_f[:, kc : kc + 1, :], wqk_ap[:, kc : kc + 1, :]
                        )
                    for g in range(1, XG):
                        nc.sync.dma_start(
                            x_ts[g][:], x_ap[:, g * NXG : (g + 1) * NXG, :]
                        )
                    wv_f = wst.tile([128, 6, 384], F32, tag="wvf")
                    nc.sync.dma_start(
                        wv_f[:], wv[:].rearrange("(c p) f -> p c f", p=128)
                    )
                    wp_f = wst.tile([128, 3, HID], F32, tag="wpf")
                    nc.sync.dma_start(
                        wp_f[:], wp[:].rearrange("(c p) f -> p c f", p=128)
                    )
                    wqk_r = wpool.tile([128, 6, 768], F32R, tag="wqkr")
                    for kc in range(6):
                        nc.vector.tensor_copy(
                            wqk_r[:, kc, :], wqk_f[:, kc, :]
                        )
                    wv_r = wpool.tile([128, 6, 384], F32R, tag="wvr")
                    nc.vector.tensor_copy(wv_r[:], wv_f[:])
                    wp_r = wpool.tile([128, 3, HID], F32R, tag="wpr")
                    nc.vector.tensor_copy(wp_r[:], wp_f[:])

                    # transposes: per x-group, per hid-chunk j, 4 seq chunks into
                    # one psum tile, then a single [128, 512] contiguous copy
                    for g in range(XG):
                        for j in range(6):
                            tp = ps.tile([128, NXG, 128], F32, tag="av", bufs=2)
                            for i in range(NXG):
                                nc.tensor.transpose(
                                    tp[:, i, :],
                                    x_ts[g][:, i, j * 128 : (j + 1) * 128],
                                    ident[:],
                                )
                            dst = xT[:, j, g * NXG * 128 : (g + 1) * NXG * 128]
                            if (g * 6 + j) % 2 == 0:
                                nc.scalar.copy(dst, tp[:])
                            else:
                                nc.vector.tensor_copy(dst, tp[:])

                if taps:
                    nc.sync.dma_start(t_xT[:], xT[:].bitcast(F32).rearrange("p a b -> p (a b)"))
                den_init = [0]
                with (
                    tc.tile_pool(name="qk", bufs=2) as qkp,
                    tc.tile_pool(name="vv", bufs=2) as vvp,
                    tc.tile_pool(name="pt", bufs=4) as ptp,
                    tc.tile_pool(name="sm", bufs=2) as smp,
                ):
                    vsl = None
                    for pj in range(PAIRS):
                        # ---- qT/kT for this pair: [128 feats, S] f32r ----
                        qTp = qkp.tile([128, S], F32R, tag="qT")
                        kTp = qkp.tile([128, S], F32R, tag="kT")
                        for n in range(NBLK):
                            for dst, wcol in ((kTp, 3 + pj), (qTp, pj)):
                                qp = ps.tile([128, 512], F32, tag="qk", bufs=1)
                                for k in range(6):
                                    nc.tensor.matmul(
                                        qp[:],
                                        wqk_r[:, k, wcol * 128 : (wcol + 1) * 128],
                                        xT[:, k, n * 512 : (n + 1) * 512],
                                        start=(k == 0),
                                        stop=(k == 5),
                                    )
                                nc.vector.tensor_scalar(
                                    dst[:, n * 512 : (n + 1) * 512],
                                    qp[:],
                                    bqk_sb[:, wcol : wcol + 1],
                                    None,
                                    ADD,
                                )

                        if pj == 0:
                            # ---- v for all 6 heads (emitted after pair-0 qkT) ----
                            vsl = vvp.tile([128, NSEQ, 6, D + 1], F32R, tag="v", bufs=1)
                            vones = smp.tile([128, NSEQ, 6, 1], F32, tag="vones")
                            nc.vector.memset(vones[:], 1.0)
                            nc.vector.tensor_copy(vsl[:, :, :, D : D + 1], vones[:])
                            for i in range(NSEQ):
                                vp = ps.tile([128, 512], F32, tag="qk", bufs=1)
                                for k in range(6):
                                    nc.tensor.matmul(
                                        vp[:, 0:384],
                                        xT[:, k, i * 128 : (i + 1) * 128],
                                        wv_r[:, k, :],
                                        start=(k == 0),
                                        stop=False,
                                    )
                                nc.tensor.matmul(
                                    vp[:, 0:384],
                                    ones_r[0:1, :],
                                    bv_r[0:1, :],
                                    start=False,
                                    stop=True,
                                )
                                nc.vector.tensor_copy(
                                    vsl[:, i, :, 0:D],
                                    vp[:, 0:384].rearrange("p (h d) -> p h d", h=6),
                                )

                        if taps and pj == 0:
                            nc.sync.dma_start(t_qT[:], qTp[:].bitcast(F32))
                            nc.sync.dma_start(t_kT[:], kTp[:].bitcast(F32))
                            nc.sync.dma_start(t_v[:], vsl[:].bitcast(F32).rearrange("p a b c -> p (a b c)"))
                        # ---- attention for the two heads of this pair ----
                        for n in range(NBLK):
                            avs = [
                                ps.tile([D + 1, 512], F32, tag="av", bufs=2, name=f"av{hi}")
                                for hi in range(2)
                            ]
                            for sk in range(NSEQ):
                                sc = ps.tile([128, 2, 512], F32, tag="sc")
                                for hi in range(2):
                                    nc.tensor.matmul(
                                        sc[:, hi, :],
                                        kTp[
                                            hi * D : (hi + 1) * D,
                                            sk * 128 : (sk + 1) * 128,
                                        ],
                                        qTp[
                                            hi * D : (hi + 1) * D,
                                            n * 512 : (n + 1) * 512,
                                        ],
                                        start=True,
                                        stop=True,
                                    )
                                pt = ptp.tile([128, 2, 512], F32R, tag="pt")
                                nc.scalar.activation(
                                    pt[:], sc[:], EXP, bias=0.0, scale=0.125
                                )
                                if taps and pj == 0 and n == 0 and sk == 0:
                                    nc.sync.dma_start(t_pt[:], pt[:].bitcast(F32).rearrange("p a b -> p (a b)"))
                                for hi in range(2):
                                    nc.tensor.matmul(
                                        avs[hi][:],
                                        vsl[:, sk, 2 * pj + hi, :],
                                        pt[:, hi, :],
                                        start=(sk == 0),
                                        stop=(sk == NSEQ - 1),
                                    )
                            # drain av psum to SBUF right away (frees the psum
                            # slot for the next block) then normalize from SBUF
                            av_sbs = []
                            for hi in range(2):
                                av_sb = smp.tile([D + 1, 512], F32, tag="avsb", name=f"av_sb{hi}")
                                nc.vector.tensor_copy(av_sb[:], avs[hi][:])
                                av_sbs.append(av_sb)
                            if taps and pj == 0 and n == 0:
                                nc.sync.dma_start(t_av[:], av_sbs[0][:])
                            den = smp.tile([33, 512], F32, tag="den")
                            if den_init[0] < 2:
                                den_init[0] += 1
                                nc.vector.memset(den[:], 1.0)
                            for hi in range(2):
                                nc.vector.tensor_copy(
                                    den[32 * hi : 32 * hi + 1, :],
                                    av_sbs[hi][D : D + 1, :],
                                )
                            rec_sb = smp.tile([33, 512], F32, tag="rec")
                            nc.vector.reciprocal(rec_sb[:], den[:])
                            rec_r = smp.tile([33, 512], F32R, tag="recr")
                            nc.vector.tensor_copy(rec_r[:], rec_sb[:])
                            for hi in range(2):
                                bc = ps.tile([D, 512], F32, tag="bc", bufs=1)
                                nc.tensor.matmul(
                                    bc[:],
                                    ones_r[32 * hi : 32 * hi + 1, 0:D],
                                    rec_r[32 * hi : 32 * hi + 1, :],
                                    start=True,
                                    stop=True,
                                )
                                if taps and pj == 0 and n == 0 and hi == 0:
                                    nc.sync.dma_start(t_rc[:], rec_r[0:1, :].bitcast(F32))
                                nc.vector.tensor_mul(
                                    aoT[
                                        hi * D : (hi + 1) * D,
                                        pj,
                                        n * 512 : (n + 1) * 512,
                                    ],
                                    av_sbs[hi][0:D, :],
                                    bc[:],
                                )

            if taps:
                nc.sync.dma_start(t_ao[:], aoT[:].bitcast(F32).rearrange("p a b -> p (a b)"))
            # ---- projection: yT[768, S] = wp^T @ aoT (partial) ----
            yT_ap = yT[:].rearrange("(c p) s -> p c s", p=128)
            with tc.tile_pool(name="yt", bufs=6) as ytp:
                for n in range(NBLK):
                    for m in range(6):
                        pp = ps.tile([128, 2, 512], F32, tag="sc")
                        for k in range(3):
                            nc.tensor.matmul(
                                pp[:, 0, :],
                                wp_r[:, k, m * 128 : (m + 1) * 128],
                                aoT[:, k, n * 512 : (n + 1) * 512],
                                start=(k == 0),
                                stop=(k == 2),
                            )
                        yt_t = ytp.tile([128, 512], F32, tag="yT")
                        if m % 2 == 0:
                            nc.scalar.copy(yt_t[:], pp[:, 0, :])
                        else:
                            nc.vector.tensor_copy(yt_t[:], pp[:, 0, :])
                        nc.sync.dma_start(
                            yT_ap[:, m, n * 512 : (n + 1) * 512], yt_t[:]
                        )

    nc.finalize()
    return nc


_NC_CACHE = {}


def _get_nc(S, taps=False):
    key = (S, taps)
    if key not in _NC_CACHE:
        _NC_CACHE[key] = build_nc(S, taps)
    return _NC_CACHE[key]


def kernel(x, qkv_w, qkv_b, proj_w, proj_b, return_res=False, **run_kwargs):
    x = np.asarray(x, dtype=np.float32)
    qkv_w = np.asarray(qkv_w, dtype=np.float32)
    qkv_b = np.asarray(qkv_b, dtype=np.float32)
    proj_w = np.asarray(proj_w, dtype=np.float32)
    proj_b = np.asarray(proj_b, dtype=np.float32)
    B, S, _ = x.shape

    nc = _get_nc(S)
    in_maps = []
    for c in range(8):
        b, g = c // 2, c % 2
        qs = slice(384 * g, 384 * g + 384)
        ks = slice(768 + 384 * g, 768 + 384 * g + 384)
        vs = slice(1536 + 384 * g, 1536 + 384 * g + 384)
        in_maps.append(
            {
                "x": np.ascontiguousarray(x[b]),
                "wqk": np.ascontiguousarray(
                    np.concatenate([qkv_w[:, qs], qkv_w[:, ks]], axis=1)
                ),
                "wv": np.ascontiguousarray(qkv_w[:, vs]),
                "bqk": np.ascontiguousarray(
                    np.concatenate([qkv_b[qs], qkv_b[ks]])
                ),
                "bv": np.ascontiguousarray(qkv_b[vs]),
                "wp": np.ascontiguousarray(proj_w[384 * g : 384 * g + 384, :]),
            }
        )
    res = run_bass_kernel_spmd(nc, in_maps, core_ids=list(range(8)), **run_kwargs)
    out = np.empty((B, S, HID), np.float32)
    for b in range(B):
        yt = res.results[2 * b]["yT"] + res.results[2 * b + 1]["yT"]
        out[b] = yt.T + proj_b
    if return_res:
        return out, res
    return out
